# revision 1
# baseline (speedup 1.0000x reference)
"""Trainium2 Bass kernel for nn_CRF_70239895159020 — lean fp16/fp32r schedule.

Reference (B=524288, C=70, 10 iterations):
    L = (S + S^T)/2 ; dL = diag(L) ; Q = log_softmax(logits, axis=1)
    repeat 10x:  P = 2*exp(Q)-1
                 tmp = logits + P @ L - dL*P       (L symmetric)
                 Q = log_sigmoid(2*tmp)

Reformulation (M = L with zero diagonal, m2 = 2M, c = colsum(M)):
    tmp2 := 2*tmp = 2*(logits + E @ m2) - 2*c   with E := exp(Q)
    E_{t+1} = sigmoid(tmp2_t),  E_0 = softmax(logits)
    output  = log_sigmoid(tmp2_9) = -ln(1 + exp(-tmp2_9))

Precision schedule: the fixed-point map is strongly contractive — a
float64 host study (calibrated against the previous fp32/fp16 kernel's
measured 7.4e-4) shows quantizing E and m2 to fp16 and the logits term
to fp32r (11-bit mantissa, fp32 range — measured by matching the HW
matmul error of rounded inputs) in EVERY iteration stays well inside the
error budget, so no fp32 iterations are kept.  Per 512-column block each
iteration is exactly 2 PE cycles/column: one fp16 matmul (m2h
stationary, E^T fp16 moving) plus one fp32r identity matmul that adds
logits^T (ACT's scale=2.0 then doubles the whole psum; fp32r streams at
1 cyc/row for N>=256, and both fp32r operands are DMA-produced, which is
what the BIR fp32r-rounding verifier requires).  logits^T is
host-transposed/permuted and DMAed directly as [C, b_core] fat
contiguous rows — no fp16 hi/lo split, no lo-correction DVE adds, no
fp32 E buffer.  ACT is the bottleneck engine (~734 us busy of ~902 us:
9 sigmoid passes + final exp + ln over a [70, 65536] state at 0.833
ns/elem); PE ~630 us; DVE ~300 us.

Phase A: softmax in natural layout (fp32), scaled into an fp16 copy,
E0^T via 1-cyc fp16 PE transposes.  Phase C: u = exp(-tmp2_9) (bf16:
fp32 exponent range, never inf — keeps the functional simulator's
finite-guard happy), clamp to 1e15 (Ln input range), v = ln(1+u) (fp16),
q = max(v, -tmp2_9) recovers the clamped region exactly (|tmp2_9| > 11
there, where log_sigmoid(x) = x to fp16), out = -q^T.

The emission is software-pipelined with prefetch distance 2 and all
cross-chunk tensors held in persistent parity-pair tiles (pool-
generation rotation is avoided on purpose: deferred cross-chunk work
would span >2 generations, which the tile framework does not protect).
Per chunk, the next chunk's iteration 0 is emitted inside this chunk's
sigmoid run, phase C at the boundary is only min/ln/max (+ the next-next
chunk's softmax, its exps pinned into the same exp/ln ACT-table
residency window via a zero bias written by phase C), and the E0 /
output transposes ride inside the next chunk's phase B where the
in-order PE queue has slack.  Measured (HW): l2 5.75e-3 (gate 2e-2),
cost-model time 901,923 ns/core vs 1,381,927 ns for the previous
fp32/fp16 kernel (1.53x).

Sharding: batch split 8 ways across cores (pure data parallel).
"""

import os
import numpy as np

B = 524288
C = 70
N_CORES = 8
B_CORE = B // N_CORES
ITERS = 10

NCH = 8192            # batch columns per chunk (transposed free dim)
BLK = 2048            # psum block columns (4 banks)

_prog_cache = {}
LAST_RESULTS = None


def build_program(b_core=B_CORE, nch=NCH, blk=BLK):
    import concourse.bass as bass
    import concourse.bacc as bacc
    import concourse.tile as tile
    from concourse import mybir
    from contextlib import ExitStack

    f32 = mybir.dt.float32
    f32r = mybir.dt.float32r
    f16 = mybir.dt.float16
    bf16 = mybir.dt.bfloat16
    AF = mybir.ActivationFunctionType
    Alu = mybir.AluOpType

    assert b_core % nch == 0
    nchunks = b_core // nch
    tpc = nch // 128
    assert nch % blk == 0 and blk % 512 == 0
    nblk = nch // blk
    gfwd = blk // 128          # fwd transposes per psum group
    gbwd = 8                   # natural slices per bwd psum group
    assert tpc % gbwd == 0

    class _Bacc(bacc.Bacc):
        # Prefer the combined exp+ln ACT table set so the final-phase
        # exp -> ln transition (and the neighbouring phase-A exp) reuses
        # one resident table instead of reloading per function.
        def insert_act_table_loads(self):
            from concourse.hw_specs import get_activation_tables
            has_act = any(isinstance(i, mybir.InstActivation)
                          for b in self.main_func.blocks
                          for i in b.instructions)
            if not has_act:
                return
            tabs = get_activation_tables(self.m.arch)
            combined = "natural_log_exp_and_others"
            AFt = mybir.ActivationFunctionType
            if combined in tabs:
                tables = [(n, (fs if n == combined else fs - {AFt.Exp, AFt.Ln}))
                          for n, fs in tabs.items()]
            else:
                tables = list(tabs.items())
            import bass_rust as _br
            _br.insert_act_table_loads(self, tables)

    nc = _Bacc("TRN2", target_bir_lowering=False)

    logits_d = nc.dram_tensor("logits", [b_core, C], f32, kind="ExternalInput")
    l1t_d = nc.dram_tensor("l1t", [C, b_core], f32r, kind="ExternalInput")
    cf32_d = nc.dram_tensor("cf32", [C, 2], f32, kind="ExternalInput")
    cr_d = nc.dram_tensor("cr", [C, C], f32r, kind="ExternalInput")
    cf16_d = nc.dram_tensor("cf16", [128, 128 + C], f16, kind="ExternalInput")
    m2h_d = nc.dram_tensor("m2h", [C, C], f16, kind="ExternalInput")
    out_d = nc.dram_tensor("out", [b_core, C], f32, kind="ExternalOutput")

    # row = k*nch + p*tpc + t: each partition reads/writes one contiguous
    # tpc*C*4B block per chunk.  The transposed-domain column order is
    # n = t*128 + p; l1t is permuted on the host to match.
    lg = logits_d[:, :].rearrange("(k p t) c -> k p t c", p=128, t=tpc)
    og = out_d[:, :].rearrange("(k p t) c -> k p t c", p=128, t=tpc)

    with tile.TileContext(nc) as tc, ExitStack() as ctx:
        const = ctx.enter_context(tc.tile_pool(name="const", bufs=1))
        state = ctx.enter_context(tc.tile_pool(name="state", bufs=1))
        stagp = ctx.enter_context(tc.tile_pool(name="stag", bufs=2))
        smallp = ctx.enter_context(tc.tile_pool(name="small", bufs=1))
        psp = ctx.enter_context(tc.tile_pool(name="ps", bufs=2, space="PSUM"))

        cf32 = const.tile([C, 2], f32)
        nc.sync.dma_start(out=cf32, in_=cf32_d[:, :])
        cr = const.tile([C, C], f32r)
        nc.sync.dma_start(out=cr, in_=cr_d[:, :])
        cf16 = const.tile([128, 128 + C], f16)
        nc.sync.dma_start(out=cf16, in_=cf16_d[:, :])
        m2ht = const.tile([C, C], f16)
        nc.sync.dma_start(out=m2ht, in_=m2h_d[:, :])
        b2sb = cf32[:, 0:1]             # -2c
        b2nsb = cf32[:, 1:2]            # +2c
        idr = cr[:, :]                  # f32r identity(70)
        idh128 = cf16[:, 0:128]         # f16 identity(128) (fwd transposes)
        m2h = m2ht[:, :]                # f16 M2
        idh = cf16[:C, 0:C]             # f16 identity(70) (bwd transposes)

        # All chunk-lived tensors are PERSISTENT tiles (allocated once; the
        # double-buffered ones as explicit parity pairs).  Pool-generation
        # rotation is deliberately avoided: with prefetch distance 2 and
        # work riding inside later chunks, accesses would span more than
        # two generations of a rotating slot, which the tile framework's
        # bookkeeping does not protect.  Stable objects make every RAW/WAR
        # visible to emission-order dependency tracking.
        natk = state.tile([128, tpc, C], f32)
        natk16 = state.tile([128, tpc, C], f16)
        l1sbP = [state.tile([C, nch], f32r, name=f"l1sb{i}") for i in (0, 1)]
        ek16P = [state.tile([C, nch], f16, name=f"ek16{i}") for i in (0, 1)]
        q16P = [state.tile([C, nch], f16, name=f"q16{i}") for i in (0, 1)]
        u16b = state.tile([C, nch], bf16)
        xn16 = state.tile([C, nch], f16)
        stag_cur = [None]     # rotating half-chunk staging (see out_group)
        zb = state.tile([128, 1], f32)                # always zero; ACT-order pin
        tc.strict_bb_all_engine_barrier()

        def phase_a_dma(k, natk_too=True):
            cols_lo = k * nch
            if natk_too:
                nc.sync.dma_start(out=natk, in_=lg[k])
            l1sb = l1sbP[k % 2]
            # group-granular loads so chunk 0's first id-matmul starts after
            # ~1/4 of the transfer
            for g in range(tpc // gfwd):
                nc.sync.dma_start(
                    out=l1sb[:, g * gfwd * 128:(g + 1) * gfwd * 128],
                    in_=l1t_d[:, cols_lo + g * gfwd * 128:
                              cols_lo + (g + 1) * gfwd * 128])

        def phase_a_softmax(k, g, pin=False):
            # softmax for one gfwd-group of columns.  zb (always zero) is
            # written early in phase C, pinning the exp into the combined
            # exp/ln ACT-table residency window so the scheduler cannot
            # scatter it into the sigmoid runs (each stray exp costs two
            # table reloads).
            import concourse.bass as _b
            gsl = slice(g * gfwd, (g + 1) * gfwd)
            natg = natk[:, gsl, :]
            if pin:
                nc.scalar.activation(natg, natg, AF.Exp, bias=zb)
            else:
                nc.scalar.activation(natg, natg, AF.Exp)
            s_t = smallp.tile([128, gfwd], f32, tag="s")
            nc.vector.reduce_sum(out=s_t, in_=natg, axis=mybir.AxisListType.X)
            r_t = smallp.tile([128, gfwd], f32, tag="r")
            nc.vector.reciprocal(out=r_t, in_=s_t)
            t1 = smallp.tile([128, gfwd], f32, tag="t1")
            nc.vector.tensor_mul(out=t1, in0=s_t, in1=r_t)
            nc.vector.tensor_scalar(out=t1, in0=t1, scalar1=-1.0, scalar2=2.0,
                                    op0=Alu.mult, op1=Alu.add)
            nc.vector.tensor_mul(out=r_t, in0=r_t, in1=t1)
            r_bcast = _b.AP(
                tensor=r_t.tensor, offset=r_t.offset,
                ap=[r_t.ap[0], r_t.ap[1], [0, C]])
            nc.vector.tensor_mul(out=natk16[:, gsl, :], in0=natg, in1=r_bcast)

        def phase_a_transpose(k, g):
            # E0^T for one group via 1-cyc fp16 PE transposes.  Emitted
            # inside the NEXT chunk's phase B so these sit behind its early
            # matmuls in the in-order PE queue instead of in front of them.
            ek16 = ek16P[k % 2]
            ptf = psp.tile([C, gfwd * 128], f16, tag="ps")
            for s in range(gfwd):
                t = g * gfwd + s
                nc.tensor.transpose(
                    ptf[:, s * 128:(s + 1) * 128], natk16[:, t, :], idh128)
            nc.vector.tensor_copy(
                out=ek16[:, g * gfwd * 128:(g + 1) * gfwd * 128], in_=ptf)

        def phase_b_iter(k, it):
            ek16 = ek16P[k % 2]
            l1sb = l1sbP[k % 2]
            last = it == ITERS - 1
            for j in range(nblk):
                pt = psp.tile([C, blk], f32, tag="ps")
                for q in range(blk // 512):
                    lo = j * blk + q * 512
                    sub = pt[:, q * 512:(q + 1) * 512]
                    nc.tensor.matmul(sub, lhsT=m2h,
                                     rhs=ek16[:, lo:lo + 512],
                                     start=True, stop=False)
                    nc.tensor.matmul(sub, lhsT=idr,
                                     rhs=l1sb[:, lo:lo + 512],
                                     start=False, stop=True)
                jsl = slice(j * blk, (j + 1) * blk)
                if not last:
                    nc.scalar.activation(ek16[:, jsl], pt, AF.Sigmoid,
                                         bias=b2sb, scale=2.0)
                else:
                    # x = 2*psum + b2; save -x (fp16) and u = exp(-x)
                    # (bf16: fp32 exponent range, never inf) per block
                    nc.vector.tensor_scalar(
                        out=xn16[:, jsl], in0=pt,
                        scalar1=-2.0, scalar2=b2nsb,
                        op0=Alu.mult, op1=Alu.add)
                    nc.scalar.activation(u16b[:, jsl], pt, AF.Exp,
                                         bias=b2nsb, scale=-2.0)

        def phase_c_compute(k):
            # v = ln(1+min(u,1e15)); q = max(v,-x).  The output transposes
            # (q^T) are deferred into the NEXT chunk's phase B (see
            # phase_c_out_group), so the boundary holds only min/ln/max.
            q16 = q16P[k % 2]
            for j in range(nblk):
                jsl = slice(j * blk, (j + 1) * blk)
                nc.vector.tensor_scalar_min(out=u16b[:, jsl], in0=u16b[:, jsl],
                                            scalar1=1e15)
                nc.scalar.activation(q16[:, jsl], u16b[:, jsl], AF.Ln,
                                     bias=1.0, scale=1.0)
                nc.vector.tensor_max(out=q16[:, jsl], in0=q16[:, jsl],
                                     in1=xn16[:, jsl])
                if j == 0:
                    # zb gated on this chunk's first ln/max via a throwaway
                    # transpose (gives it 128 partitions)
                    pz = psp.tile([128, C], f16, tag="ps")
                    nc.tensor.transpose(pz, q16[:, 0:128], idh)
                    nc.vector.tensor_scalar_mul(out=zb, in0=pz[:, 0:1],
                                                scalar1=0.0)

        def phase_c_out_group(k, g):
            # one gbwd-group of output transposes + negate-stage; emitted
            # inside the next chunk's phase B where q16 is long ready, so
            # these PE slices never gate the boundary.  stagk holds half a
            # chunk; it is stored (and reused) twice per chunk.
            q16 = q16P[k % 2]
            pn = psp.tile([128, gbwd * C], f16, tag="ps")
            for s in range(gbwd):
                t = g * gbwd + s
                nc.tensor.transpose(
                    pn[:, s * C:(s + 1) * C],
                    q16[:, t * 128:(t + 1) * 128], idh)
            ngh_ = tpc // (2 * gbwd)
            gh = g % ngh_
            if gh == 0:
                # fresh pool generation per half: allocation at first use
                # keeps pool-generation distance <= 1, the regime the tile
                # framework's rotation bookkeeping actually protects
                stag_cur[0] = stagp.tile([128, tpc // 2, C], f32,
                                         name="stagh", tag="stag")
            nc.vector.tensor_scalar_mul(
                out=stag_cur[0][:, gh * gbwd:(gh + 1) * gbwd, :],
                in0=pn.rearrange("p (a c) -> p a c", c=C),
                scalar1=-1.0)

        def phase_c_out_store(k, half):
            h = tpc // 2
            nc.sync.dma_start(out=og[k][:, half * h:(half + 1) * h, :],
                              in_=stag_cur[0])

        # Software-pipelined emission, prefetch distance 2.  Within loop k:
        #   0. natk(k+2) DMA (runs during B(k))
        #   1. B(k) iters 1..8; riding along: A(k+1)'s E0 transposes
        #      (iters 3..6) and C(k-1)'s output transposes + stores
        #   2. chunk k+1's iteration 0 (sigmoids merge with the sigmoid run)
        #   3. B(k) iteration 9 (the exp that feeds phase C)
        #   4. l1sb(k+2) DMA (reuses the l1sb slot it9 just finished with)
        #   5. C(k)-compute: min/ln/max (+ zb pin write)
        #   6. A(k+2)'s softmax (exp pinned next to C's exp/ln table window)
        ngr = tpc // gfwd
        ngb = tpc // gbwd           # deferred output-transpose groups
        ngh = ngb // 2
        phase_a_dma(0)
        for g in range(ngr):
            phase_a_softmax(0, g)
            phase_a_transpose(0, g)
        if nchunks > 1:
            phase_a_dma(1)
        phase_b_iter(0, 0)
        for k in range(nchunks):
            if k >= 1 and k + 2 < nchunks:
                # safe from k>=1: A(k+1)'s softmax (the previous natk
                # reader) was emitted last boundary.  At k==0 chunk 1's
                # softmax still rides inside B(0), so the overwrite waits.
                nc.sync.dma_start(out=natk, in_=lg[k + 2])
            for it in range(1, ITERS - 1):
                phase_b_iter(k, it)
                if k == 0 and nchunks > 1 and it - 1 < ngr:
                    phase_a_softmax(1, it - 1)
                    phase_a_transpose(1, it - 1)
                elif k >= 1 and k + 1 < nchunks and 0 <= it - 3 < ngr:
                    # E0 transposes for chunk k+1 (softmax ran last boundary)
                    phase_a_transpose(k + 1, it - 3)
                if k >= 1 and it - 1 < ngb:
                    phase_c_out_group(k - 1, it - 1)
                    if it - 1 == ngh - 1:
                        phase_c_out_store(k - 1, 0)
            if k >= 1:
                phase_c_out_store(k - 1, 1)
            if k + 1 < nchunks:
                phase_b_iter(k + 1, 0)
            phase_b_iter(k, ITERS - 1)
            if k + 2 < nchunks:
                phase_a_dma(k + 2, natk_too=(k == 0))
            phase_c_compute(k)
            if k + 2 < nchunks:
                for g in range(ngr):
                    phase_a_softmax(k + 2, g, pin=True)
        # last chunk's output transposes have no next B to ride in
        for g in range(ngb):
            phase_c_out_group(nchunks - 1, g)
            if g == ngh - 1:
                phase_c_out_store(nchunks - 1, 0)
        phase_c_out_store(nchunks - 1, 1)

    nc.compile()
    return nc


def _host_prep(logits, similarities):
    S = np.asarray(similarities, dtype=np.float32)
    L = (S + S.T) * np.float32(0.5)
    M = L.copy()
    np.fill_diagonal(M, 0.0)
    m2 = (2.0 * M).astype(np.float32)
    col = M.astype(np.float64).sum(axis=0)
    cf32 = np.zeros((C, 2), dtype=np.float32)
    cf32[:, 0] = (-2.0 * col).astype(np.float32)
    cf32[:, 1] = (2.0 * col).astype(np.float32)
    cr = np.eye(C, dtype=np.float32)
    cf16 = np.zeros((128, 128 + C), dtype=np.float16)
    cf16[:, 0:128] = np.eye(128, dtype=np.float16)
    cf16[:C, 128:128 + C] = m2.astype(np.float16)
    m2h = m2.astype(np.float16)

    # device ek column n (of chunk k) holds batch row k*NCH + p*TPC + t
    # where n = t*128 + p; permute rows to device column order, then
    # transpose so each l1t row (one label c) is contiguous.
    tpc = NCH // 128

    def perm(a):
        b_all, c = a.shape
        v = a.reshape(b_all // NCH, 128, tpc, c)           # [k, p, t, c]
        v = np.ascontiguousarray(v.transpose(0, 2, 1, 3))  # [k, t, p, c]
        return v.reshape(b_all, c)

    l1t = np.ascontiguousarray(perm(logits).T)             # [C, B]
    return cf32, cr, cf16, m2h, l1t


def kernel(logits, similarities):
    global LAST_RESULTS
    from concourse.bass_utils import run_bass_kernel_spmd

    logits = np.ascontiguousarray(np.asarray(logits), dtype=np.float32)
    cf32, cr, cf16, m2h, l1t = _host_prep(logits, similarities)

    key = (B_CORE, NCH, BLK)
    if key not in _prog_cache:
        _prog_cache[key] = build_program()
    nc = _prog_cache[key]

    shards = logits.reshape(N_CORES, B_CORE, C)
    l1t_s = l1t.reshape(C, N_CORES, B_CORE)
    in_maps = []
    for i in range(N_CORES):
        m = {"logits": shards[i],
             "l1t": np.ascontiguousarray(l1t_s[:, i, :]),
             "cf32": cf32, "cr": cr, "cf16": cf16, "m2h": m2h}
        in_maps.append(m)
    trace = os.environ.get("KERNEL_TRACE", "0") == "1"
    res = run_bass_kernel_spmd(nc, in_maps, core_ids=list(range(N_CORES)),
                               trace=trace)
    LAST_RESULTS = res
    out = np.concatenate([r["out"] for r in res.results], axis=0)
    return np.ascontiguousarray(out, dtype=np.float32)

